# revision 1
# baseline (speedup 1.0000x reference)
"""GAT layer (multi-head graph attention) on 8 TRN2 NeuronCores.

Strategy (per sharding hint): destination nodes are sharded across the 8
cores.  Each core:
  phase 1: computes the full projection table redundantly (bf16 GEMM
           X @ W.T plus the per-head attention score reductions), packed
           as [proj bf16 | s_src f32 | s_tgt f32 | pad] rows in local HBM.
  phase 2: walks its shard's destination windows (128 targets / window).
           Edges are pre-sorted by (window, src-bucket) on the host;
           dma_gather pulls the source rows (int16 indices per 32768-row
           bucket), scores -> leaky-relu -> exp run batched per window,
           and one-hot matmuls (host-streamed) accumulate both the
           softmax denominator and the weighted aggregation in PSUM.
           Softmax division + PReLU happen once per window at flush.

kernel(**inputs) takes the FULL inputs and returns the FULL output.
"""

import math
from dataclasses import dataclass, field

import numpy as np
import ml_dtypes

BF16 = ml_dtypes.bfloat16
P = 128


def _ceil(a, b):
    return -(-a // b)


@dataclass
class Cfg:
    N: int = 100000
    E: int = 800000
    HID: int = 512
    HEADS: int = 8
    ncores: int = 8
    bucket: int = 32768
    leak: float = 0.01
    oh_bf16: bool = True  # one-hot stream dtype (bf16; fp8 is an option)

    def __post_init__(self):
        assert self.N % self.ncores == 0
        assert self.bucket <= 32768
        self.F = self.HID // self.HEADS
        self.shard = self.N // self.ncores
        self.NW = _ceil(self.shard, P)          # windows per core
        self.NB = _ceil(self.N, self.bucket)    # src buckets (int16 range)
        self.NT = _ceil(self.N, P)              # projection tiles
        self.NPAD = self.NT * P
        self.KP = min(self.HID, P)              # contraction partitions
        self.KT = self.HID // self.KP           # contraction tiles
        row_bytes = self.HID * 2 + 2 * self.HEADS * 4
        self.row_bytes = _ceil(row_bytes, 256) * 256
        self.row_bf = self.row_bytes // 2
        self.row_f32 = self.row_bytes // 4
        self.s_src_off = self.HID // 2          # f32 col of s_src in a row
        self.s_tgt_off = self.HID // 2 + self.HEADS


@dataclass
class Schedule:
    """Core-independent (uniform) phase-2 schedule."""
    seg: np.ndarray          # [NW, NB] slot counts (128-aligned, global max)
    TW: list                 # tiles per window
    TWmax: int
    calls: list              # per window: list of (b, slot_off, nslots, idxcol0)
    idxcols: int             # total int16 idx columns (per 16-wrap row)
    TT: int                  # total tiles
    tile_base: list          # first global tile index of each window


def build_schedule(cfg: Cfg, counts: np.ndarray) -> Schedule:
    """counts: [ncores, NW, NB] edge counts."""
    maxcnt = counts.max(axis=0)  # [NW, NB]
    seg = np.where(maxcnt > 0, _ceil(maxcnt, P) * P, 0).astype(np.int64)
    TW, calls, tile_base = [], [], []
    idxcol = 0
    tt = 0
    for w in range(cfg.NW):
        tile_base.append(tt)
        wcalls = []
        off = 0
        for b in range(cfg.NB):
            s = int(seg[w, b])
            if s == 0:
                continue
            wcalls.append((b, off, s, idxcol))
            off += s
            idxcol += s // 16
        assert off % P == 0
        TW.append(off // P)
        tt += off // P
        calls.append(wcalls)
    return Schedule(seg=seg, TW=TW, TWmax=max(TW), calls=calls,
                    idxcols=idxcol, TT=tt, tile_base=tile_base)


def prep_core(cfg: Cfg, sched: Schedule, src, trg, k):
    """Per-core input arrays: g1 idx stream and one-hot stream."""
    oh_dt = BF16 if cfg.oh_bf16 else ml_dtypes.float8_e4m3
    mask = (trg // cfg.shard) == k
    esrc = src[mask]
    etrg = trg[mask]
    trel = etrg - k * cfg.shard
    win = trel // P
    buck = esrc // cfg.bucket
    # order edges by (window, bucket); stable so host/device agree
    order = np.lexsort((buck, win))
    esrc, etrg, trel, win, buck = (a[order] for a in (esrc, etrg, trel, win, buck))

    g1i = np.zeros((P, sched.idxcols), np.int16)
    oh = np.zeros((P, sched.TT, 2, P), oh_dt)

    # per (window, bucket) segment boundaries
    key = win * cfg.NB + buck
    # edge ranges per (w, b)
    starts = np.searchsorted(key, np.arange(cfg.NW * cfg.NB), side="left")
    ends = np.searchsorted(key, np.arange(cfg.NW * cfg.NB), side="right")

    for w in range(cfg.NW):
        for (b, slot_off, nslots, idxcol0) in sched.calls[w]:
            lo, hi = int(starts[w * cfg.NB + b]), int(ends[w * cfg.NB + b])
            cnt = hi - lo
            assert cnt <= nslots
            idx = np.zeros(nslots, np.int16)
            idx[:cnt] = (esrc[lo:hi] - b * cfg.bucket).astype(np.int16)
            blk = idx.reshape(nslots // 16, 16).T          # [16, cols]
            g1i[:, idxcol0:idxcol0 + nslots // 16] = np.tile(blk, (8, 1))
            # one-hots for this segment's tiles
            tloc = (trel[lo:hi] - w * P).astype(np.int64)  # [cnt] in [0,128)
            t0 = sched.tile_base[w] + slot_off // P
            for j in range(nslots // P):
                s0, s1 = j * P, min((j + 1) * P, cnt)
                if s1 <= s0:
                    continue
                rows = np.arange(s0, s1) - s0
                cols = tloc[s0:s1]
                oh[rows, t0 + j, 0, cols] = oh_dt(1.0)
                oh[cols, t0 + j, 1, rows] = oh_dt(1.0)
    return g1i, oh


def pack_xt(cfg: Cfg, X: np.ndarray) -> np.ndarray:
    """X [N, HID] f32 -> bf16 packed [KP, NT, KT, P]: (p, j, ki, n) = X[j*P+n, ki*KP+p]."""
    Xp = np.zeros((cfg.NPAD, cfg.HID), np.float32)
    Xp[: cfg.N] = X
    Xb = Xp.astype(BF16)
    # [NT, P(n), KT, KP(p)] -> transpose to [KP, NT, KT, P]
    v = Xb.reshape(cfg.NT, P, cfg.KT, cfg.KP)
    return np.ascontiguousarray(v.transpose(3, 0, 2, 1))


def pack_w(cfg: Cfg, W, a_src, a_tgt):
    """Returns wt [KP, KT, HID] bf16 and wa [KP, KT, 2*HEADS] bf16."""
    WT = W.T.astype(np.float32)                       # [HID(d), HID(o)]
    wa_s = (W.reshape(cfg.HEADS, cfg.F, cfg.HID)
            * np.asarray(a_src, np.float32).reshape(cfg.HEADS, cfg.F, 1)).sum(1)  # [H, d]
    wa_t = (W.reshape(cfg.HEADS, cfg.F, cfg.HID)
            * np.asarray(a_tgt, np.float32).reshape(cfg.HEADS, cfg.F, 1)).sum(1)
    WA = np.concatenate([wa_s.T, wa_t.T], axis=1)     # [d, 2H]
    wt = np.ascontiguousarray(
        WT.astype(BF16).reshape(cfg.KT, cfg.KP, cfg.HID).transpose(1, 0, 2))
    wa = np.ascontiguousarray(
        WA.astype(BF16).reshape(cfg.KT, cfg.KP, 2 * cfg.HEADS).transpose(1, 0, 2))
    return wt, wa


def _bcast_last(ap, n):
    """Append a 0-stride broadcast dim of size n to an AP."""
    import concourse.bass as bass
    lst = [list(x) for x in ap.ap] + [[0, n]]
    return bass.AP(ap.tensor, ap.offset, lst)


def build_nc(cfg: Cfg, sched: Schedule, phases: str = "full"):
    import concourse.bacc as bacc
    import concourse.bass as bass
    import concourse.mybir as mybir
    from concourse.tile import TileContext

    dt = mybir.dt
    oh_mdt = dt.bfloat16 if cfg.oh_bf16 else dt.float8e4
    H, HID, KT, KP = cfg.HEADS, cfg.HID, cfg.KT, cfg.KP

    nc = bacc.Bacc("TRN2", target_bir_lowering=False)

    xt = nc.dram_tensor("xt", [KP, cfg.NT, KT, P], dt.bfloat16, kind="ExternalInput")
    wt = nc.dram_tensor("wt", [KP, KT, HID], dt.bfloat16, kind="ExternalInput")
    wa = nc.dram_tensor("wa", [KP, KT, 2 * H], dt.bfloat16, kind="ExternalInput")
    g1i = nc.dram_tensor("g1i", [P, sched.idxcols], dt.int16, kind="ExternalInput")
    ohd = nc.dram_tensor("ohd", [P, sched.TT, 2, P], oh_mdt, kind="ExternalInput")
    avec = nc.dram_tensor("avec", [P, 1], dt.float32, kind="ExternalInput")
    out = nc.dram_tensor("out", [cfg.NW * P, HID], dt.float32, kind="ExternalOutput")

    with TileContext(nc) as tc:
        with tc.tile_pool(name="const", bufs=1) as cpool, \
             tc.tile_pool(name="dram", bufs=1, space="DRAM") as dpool:
            table = dpool.tile([cfg.NPAD, cfg.row_bf], dt.bfloat16)
            wt_sb = cpool.tile([KP, KT, HID], dt.bfloat16)
            nc.sync.dma_start(out=wt_sb[:], in_=wt[:, :, :])
            wa_sb = cpool.tile([KP, KT, 2 * H], dt.bfloat16)
            nc.sync.dma_start(out=wa_sb[:], in_=wa[:, :, :])
            if phases == "full":
                a_sb = cpool.tile([P, 1], dt.float32)
                nc.sync.dma_start(out=a_sb[:], in_=avec[:, :])
            if phases in ("full", "p1g"):
                g1i_sb = cpool.tile([P, sched.idxcols], dt.int16)
                nc.sync.dma_start(out=g1i_sb[:], in_=g1i[:, :])

            # ---------------- phase 1: projection table ----------------
            with tc.tile_pool(name="p1", bufs=3) as xpool, \
                 tc.tile_pool(name="p1ps", bufs=2, space="PSUM") as pspool, \
                 tc.tile_pool(name="p1st", bufs=3) as stpool:
                for j in range(cfg.NT):
                    xtile = xpool.tile([KP, KT, P], dt.bfloat16, tag="x")
                    nc.sync.dma_start(out=xtile[:], in_=xt[:, j, :, :])
                    ps1 = pspool.tile([P, HID], dt.float32, tag="ps1")
                    ps2 = pspool.tile([P, 2 * H], dt.float32, tag="ps2")
                    for ki in range(KT):
                        nc.tensor.matmul(ps1[:], xtile[:, ki, :], wt_sb[:, ki, :],
                                         start=(ki == 0), stop=(ki == KT - 1))
                    for ki in range(KT):
                        nc.tensor.matmul(ps2[:], xtile[:, ki, :], wa_sb[:, ki, :],
                                         start=(ki == 0), stop=(ki == KT - 1))
                    stg = stpool.tile([P, cfg.row_bf], dt.bfloat16, tag="stg")
                    stg32 = stg.bitcast(dt.float32)
                    nc.scalar.copy(out=stg[:, 0:HID], in_=ps1[:])
                    nc.scalar.copy(out=stg32[:, cfg.s_src_off:cfg.s_src_off + 2 * H],
                                   in_=ps2[:])
                    if cfg.s_tgt_off + H < cfg.row_f32:
                        nc.vector.memset(stg32[:, cfg.s_tgt_off + H:cfg.row_f32], 0.0)
                    nc.sync.dma_start(out=table[j * P:(j + 1) * P, :], in_=stg[:])

            tc.strict_bb_all_engine_barrier()

            if phases == "p1":
                with tc.tile_pool(name="dbg", bufs=2) as dbgp:
                    for w in range(cfg.NW):
                        res = dbgp.tile([P, HID], dt.float32, tag="res")
                        nc.vector.memset(res[:], 0.0)
                        nc.sync.dma_start(out=out[w * P:(w + 1) * P, :], in_=res[:])
                nc.compile()
                return nc

            if phases == "p1g":
                with tc.tile_pool(name="dbg", bufs=2) as dbgp:
                    for w in range(cfg.NW):
                        g1t = dbgp.tile([P, sched.TWmax, cfg.row_bf], dt.bfloat16,
                                        tag="g1t")
                        for (b, slot_off, nslots, idxcol0) in sched.calls[w]:
                            rows = min(cfg.NPAD, (b + 1) * cfg.bucket) - b * cfg.bucket
                            nc.gpsimd.dma_gather(
                                g1t[:, slot_off // P:(slot_off + nslots) // P, :],
                                table[b * cfg.bucket:b * cfg.bucket + rows, :],
                                g1i_sb[:, idxcol0:idxcol0 + nslots // 16],
                                nslots, nslots, cfg.row_bf)
                        res = dbgp.tile([P, HID], dt.float32, tag="res")
                        nc.vector.memset(res[:], 0.0)
                        nc.sync.dma_start(out=out[w * P:(w + 1) * P, :], in_=res[:])
                nc.compile()
                return nc

            # ---------------- phase 1.5: resident s_tgt (hi/lo bf16) ----------------
            pid = nc.sync.partition_id()
            table32 = table.bitcast(dt.float32)
            s_ap = table32[bass.DynSlice(pid * cfg.shard, cfg.NW * P),
                           cfg.s_tgt_off:cfg.s_tgt_off + H]
            s_ap = s_ap.rearrange("(w p) h -> p w h", p=P)
            s_all = cpool.tile([P, cfg.NW, H], dt.float32)
            nc.sync.dma_start(out=s_all[:], in_=s_ap)
            s_hilo = cpool.tile([P, cfg.NW, 2, H], dt.bfloat16)
            s_hi32 = cpool.tile([P, cfg.NW, H], dt.float32)
            nc.vector.tensor_copy(out=s_hilo[:, :, 0, :], in_=s_all[:])
            nc.vector.tensor_copy(out=s_hi32[:], in_=s_hilo[:, :, 0, :])
            nc.vector.tensor_tensor(out=s_hilo[:, :, 1, :], in0=s_all[:],
                                    in1=s_hi32[:], op=mybir.AluOpType.subtract)

            # ---------------- phase 2: windows ----------------
            with tc.tile_pool(name="p2", bufs=2) as pool, \
                 tc.tile_pool(name="p2ps", bufs=2, space="PSUM") as pps:
                for w in range(cfg.NW):
                    Tw = sched.TW[w]
                    g1t = pool.tile([P, sched.TWmax, cfg.row_bf], dt.bfloat16, tag="g1t")
                    for (b, slot_off, nslots, idxcol0) in sched.calls[w]:
                        rows = min(cfg.NPAD, (b + 1) * cfg.bucket) - b * cfg.bucket
                        nc.gpsimd.dma_gather(
                            g1t[:, slot_off // P:(slot_off + nslots) // P, :],
                            table[b * cfg.bucket:b * cfg.bucket + rows, :],
                            g1i_sb[:, idxcol0:idxcol0 + nslots // 16],
                            nslots, nslots, cfg.row_bf)
                    jb = sched.tile_base[w]
                    oht = pool.tile([P, sched.TWmax, 2, P], oh_mdt, tag="oht")
                    nc.sync.dma_start(out=oht[:, :Tw, :, :], in_=ohd[:, jb:jb + Tw, :, :])

                    # s_tgt expansion (per tile) via transposed one-hot matmul
                    stgt = pps.tile([P, sched.TWmax, 2, H], dt.float32, tag="stgt")
                    for t in range(Tw):
                        nc.tensor.matmul(stgt[:, t, :, :], oht[:, t, 1, :],
                                         s_hilo[:, w, :, :], start=True, stop=True)
                    g1t32 = g1t.bitcast(dt.float32)
                    s_sum = pool.tile([P, sched.TWmax, H], dt.float32, tag="s_sum")
                    s_act = pool.tile([P, sched.TWmax, H], dt.float32, tag="s_act")
                    nc.vector.tensor_tensor(
                        out=s_sum[:, :Tw, :], in0=stgt[:, :Tw, 0, :],
                        in1=g1t32[:, :Tw, cfg.s_src_off:cfg.s_src_off + H],
                        op=mybir.AluOpType.add)
                    nc.vector.tensor_tensor(
                        out=s_act[:, :Tw, :], in0=stgt[:, :Tw, 1, :],
                        in1=s_sum[:, :Tw, :], op=mybir.AluOpType.add)
                    nc.vector.scalar_tensor_tensor(
                        out=s_sum[:, :Tw, :], in0=s_act[:, :Tw, :], scalar=cfg.leak,
                        in1=s_act[:, :Tw, :], op0=mybir.AluOpType.mult,
                        op1=mybir.AluOpType.max)
                    exp_t = pool.tile([P, sched.TWmax, H], dt.bfloat16, tag="exp_t")
                    nc.scalar.activation(out=exp_t[:, :Tw, :], in_=s_sum[:, :Tw, :],
                                         func=mybir.ActivationFunctionType.Exp)

                    w_t = pool.tile([P, sched.TWmax, HID], dt.bfloat16, tag="w_t")
                    proj4 = g1t[:, :Tw, 0:HID].rearrange("p t (h f) -> p t h f", h=H)
                    exp4 = _bcast_last(exp_t[:, :Tw, :], cfg.F)
                    out4 = w_t[:, :Tw, :].rearrange("p t (h f) -> p t h f", h=H)
                    nc.vector.tensor_tensor(out=out4, in0=proj4, in1=exp4,
                                            op=mybir.AluOpType.mult)

                    agg = pps.tile([P, HID], dt.float32, tag="agg")
                    den = pps.tile([P, H], dt.float32, tag="den")
                    for t in range(Tw):
                        nc.tensor.matmul(agg[:], oht[:, t, 0, :], w_t[:, t, :],
                                         start=(t == 0), stop=(t == Tw - 1))
                        nc.tensor.matmul(den[:], oht[:, t, 0, :], exp_t[:, t, :],
                                         start=(t == 0), stop=(t == Tw - 1))

                    # flush: softmax divide + PReLU
                    den_sb = pool.tile([P, H, 1], dt.float32, tag="den_sb")
                    recip = pool.tile([P, H, 1], dt.float32, tag="recip")
                    nc.vector.tensor_scalar_add(out=den_sb[:, :, 0], in0=den[:],
                                                scalar1=1e-16)
                    nc.vector.reciprocal(out=recip[:], in_=den_sb[:])
                    z = pool.tile([P, HID], dt.float32, tag="z")
                    agg4 = agg[:].rearrange("p (h f) -> p h f", h=H)
                    z4 = z[:].rearrange("p (h f) -> p h f", h=H)
                    nc.vector.tensor_tensor(out=z4, in0=agg4,
                                            in1=_bcast_last(recip[:, :, 0], cfg.F),
                                            op=mybir.AluOpType.mult)
                    res = pool.tile([P, HID], dt.float32, tag="res")
                    nc.vector.scalar_tensor_tensor(
                        out=res[:], in0=z[:], scalar=a_sb[:, 0:1], in1=z[:],
                        op0=mybir.AluOpType.mult, op1=mybir.AluOpType.max)
                    nc.sync.dma_start(out=out[w * P:(w + 1) * P, :], in_=res[:])

    nc.compile()
    return nc


def prepare(cfg: Cfg, inputs):
    """Host-side prep shared by HW and sim paths.

    Returns (sched, in_maps, assemble) where assemble(core_outs) -> full out.
    """
    X = np.asarray(inputs["in_nodes_features"], np.float32)
    ei = np.asarray(inputs["edge_index"], np.int64)
    W = np.asarray(inputs["W"], np.float32)
    b_lin = np.asarray(inputs["b_lin"], np.float32)
    a_src = np.asarray(inputs["a_src"], np.float32)
    a_tgt = np.asarray(inputs["a_tgt"], np.float32)
    bias = np.asarray(inputs["bias"], np.float32)
    prelu_a = float(np.asarray(inputs["prelu_a"], np.float32))

    assert np.all(b_lin == 0) and np.all(bias == 0), "nonzero bias unsupported"
    assert 0.0 <= prelu_a <= 1.0, "prelu_a outside [0,1] unsupported"

    src, trg = ei[0], ei[1]
    core_of = trg // cfg.shard
    win_of = (trg % cfg.shard) // P
    buck_of = src // cfg.bucket
    counts = np.zeros((cfg.ncores, cfg.NW, cfg.NB), np.int64)
    for k in range(cfg.ncores):
        m = core_of == k
        counts[k] = np.bincount(
            win_of[m] * cfg.NB + buck_of[m],
            minlength=cfg.NW * cfg.NB).reshape(cfg.NW, cfg.NB)
    sched = build_schedule(cfg, counts)

    xt = pack_xt(cfg, X)
    wtp, wap = pack_w(cfg, W, a_src, a_tgt)
    av = np.full((P, 1), prelu_a, np.float32)

    in_maps = []
    for k in range(cfg.ncores):
        g1i_k, oh_k = prep_core(cfg, sched, src, trg, k)
        in_maps.append({
            "xt": xt, "wt": wtp, "wa": wap,
            "g1i": g1i_k, "ohd": oh_k, "avec": av,
        })

    def assemble(core_outs):
        return np.concatenate(
            [np.asarray(o["out"][: cfg.shard], np.float32) for o in core_outs], axis=0)

    return sched, in_maps, assemble


_BUILT = {}


def _get_built(cfg: Cfg, sched: Schedule):
    key = (cfg.N, cfg.E, cfg.HID, cfg.HEADS, cfg.ncores, cfg.bucket,
           tuple(sched.TW), sched.idxcols)
    if key not in _BUILT:
        _BUILT[key] = build_nc(cfg, sched)
    return _BUILT[key]


def kernel(**inputs):
    from concourse.bass_utils import run_bass_kernel_spmd

    cfg = Cfg()
    sched, in_maps, assemble = prepare(cfg, inputs)
    nc = _get_built(cfg, sched)
    res = run_bass_kernel_spmd(nc, in_maps, core_ids=list(range(cfg.ncores)))
    return assemble(res.results)



# revision 15
# speedup vs baseline: 1.2024x; 1.2024x over previous
"""GAT layer (multi-head graph attention) on 8 TRN2 NeuronCores — V2.

Structure (vs V1 baseline):
  phase 1 : projection GEMM sharded 8-way by node rows (98 tiles/core,
            per-core xt input slice); rows packed [proj bf16 | s_src f32 |
            s_tgt f32 | pad] and written to a local DRAM shard.
  CC      : one 8-core AllGather assembles the full 100352-row table
            (pair-shared HBM output).  (mode="pair": each core instead
            computes half the table into pair-shared HBM, tiny AllReduce
            as a fence.)
  phase 2 : destination windows, processed in groups of G=2 windows.
            Per group: 4 dma_gather calls (one per 32768-row src bucket),
            the fwd one-hot is generated on device (tloc vs iota compare),
            the rev one-hot streams from host; s_tgt expansion + softmax
            chain run group-wide; agg/den accumulate in PSUM banks per
            window (w-major matmul order).
"""

import numpy as np
import ml_dtypes

BF16 = ml_dtypes.bfloat16
P = 128


def _ceil(a, b):
    return -(-a // b)


class Cfg2:
    def __init__(self, fp8: bool = False):
        self.fp8 = fp8
        self.N = 100000
        self.E = 800000
        self.HID = 512
        self.HEADS = 8
        self.F = self.HID // self.HEADS
        self.ncores = 8
        self.G = 2
        self.leak = 0.01
        self.bucket = 32768
        self.NTC = 98                      # phase-1 tiles per core
        self.shard = self.NTC * P          # 12544 rows per core
        self.NPAD8 = self.ncores * self.shard   # 100352
        self.NT = self.NPAD8 // P          # 784
        self.NB = _ceil(self.NPAD8, self.bucket)  # 4
        self.NW = self.NTC                 # 98 windows per core
        self.NG = self.NW // self.G        # 49 groups
        # quarter split of each core's shard (tile-aligned) for chunked CC;
        # uneven split keeps gather buckets at [32768,32768,32768,2048] rows
        # (same slot padding as a single CC) while letting CC_q pipeline
        # behind phase-1 quarter q.
        self.qtiles = [32, 32, 32, 2]
        self.qrows = [t * P for t in self.qtiles]           # per-core rows
        self.qstart = np.cumsum([0] + self.qrows)[:-1]      # within-shard row
        self.qtstart = np.cumsum([0] + self.qtiles)[:-1]    # within-shard tile
        self.brows = [self.ncores * r for r in self.qrows]  # bucket rows
        self.KP = min(self.HID, P)
        self.KT = self.HID // self.KP
        self.proj_bytes = self.HID * (1 if fp8 else 2)
        row_bytes = self.proj_bytes + 2 * self.HEADS * 4
        self.row_bytes = _ceil(row_bytes, 256) * 256   # 768 fp8 / 1280 bf16
        self.row_bf = self.row_bytes // 2
        self.row_f32 = self.row_bytes // 4
        self.s_src_off = self.proj_bytes // 4          # f32 units
        self.s_tgt_off = self.s_src_off + self.HEADS


class Sched2:
    """Uniform (core-independent) grouped phase-2 schedule."""

    def __init__(self, cfg: Cfg2, counts: np.ndarray):
        # counts: [ncores, NW, NB]
        maxcnt = counts.max(axis=0)
        self.seg = np.where(maxcnt > 0, _ceil(maxcnt, P) * P, 0).astype(np.int64)
        self.TG = []          # tiles per group
        self.calls = []       # per group: (b, slot_off, nslots, idxcol0)
        self.wsel = []        # per group: window-in-group of each tile
        self.tslot = []       # per group: first slot of each tile
        self.tile_base = []   # first global tile index of each group
        idxcol = 0
        tt = 0
        for g in range(cfg.NG):
            ws = [cfg.G * g + i for i in range(cfg.G)]
            gcalls, gwsel, gtslot = [], [], []
            off = 0
            for b in range(cfg.NB):
                ns = int(sum(self.seg[w, b] for w in ws))
                if ns == 0:
                    continue
                gcalls.append((b, off, ns, idxcol))
                for wi, w in enumerate(ws):
                    s = int(self.seg[w, b])
                    for j in range(s // P):
                        gwsel.append(wi)
                        gtslot.append(off + sum(int(self.seg[w2, b])
                                                for w2 in ws[:wi]) + j * P)
                off += ns
                idxcol += ns // 16
            assert off % P == 0
            self.tile_base.append(tt)
            self.TG.append(off // P)
            tt += off // P
            self.calls.append(gcalls)
            self.wsel.append(gwsel)
            self.tslot.append(gtslot)
        self.TGmax = max(self.TG)
        self.TT = tt
        self.idxcols = idxcol


def prep_core2(cfg: Cfg2, sched: Sched2, eidx, ebuck, trg, k):
    """Per-core streams: g1i idx wrap, tlocP, ohrev."""
    mask = (trg // cfg.shard) == k
    esrc = eidx[mask]
    etrg = trg[mask]
    trel = etrg - k * cfg.shard
    win = trel // P
    buck = ebuck[mask]
    order = np.lexsort((esrc, buck, win))  # src-sorted: ascending gather addrs
    esrc, trel, win, buck = (a[order] for a in (esrc, trel, win, buck))

    key = win * cfg.NB + buck
    starts = np.searchsorted(key, np.arange(cfg.NW * cfg.NB), side="left")
    ends = np.searchsorted(key, np.arange(cfg.NW * cfg.NB), side="right")

    g1i = np.zeros((P, sched.idxcols), np.int16)
    tlocP = np.full((P, sched.TT), 200.0, BF16)
    ohrev = np.zeros((P, sched.TT, P), BF16)

    for g in range(cfg.NG):
        ws = [cfg.G * g + i for i in range(cfg.G)]
        tbase = sched.tile_base[g]
        tof = 0  # tile offset within group
        for (b, slot_off, nslots, idxcol0) in sched.calls[g]:
            idx = np.zeros(nslots, np.int16)
            soff = 0
            for w in ws:
                s = int(sched.seg[w, b])
                if s == 0:
                    continue
                lo, hi = int(starts[w * cfg.NB + b]), int(ends[w * cfg.NB + b])
                cnt = hi - lo
                assert cnt <= s
                idx[soff:soff + cnt] = esrc[lo:hi].astype(np.int16)
                # per-tile tloc / rev one-hot
                tl = (trel[lo:hi] - w * P).astype(np.int64)
                for j in range(s // P):
                    t = tbase + tof
                    s0, s1 = j * P, min((j + 1) * P, cnt)
                    if s1 > s0:
                        rows = np.arange(s0, s1) - s0
                        cols = tl[s0:s1]
                        tlocP[rows, t] = cols.astype(BF16)
                        ohrev[cols, t, rows] = BF16(1.0)
                    tof += 1
                soff += s
            blk = idx.reshape(nslots // 16, 16).T
            g1i[:, idxcol0:idxcol0 + nslots // 16] = np.tile(blk, (8, 1))
    return g1i, tlocP, ohrev


def pack_xt2(cfg: Cfg2, X: np.ndarray):
    """X [N, HID] f32 -> per-core bf16 [KP, NTC, KT, P] slices."""
    Xp = np.zeros((cfg.NPAD8, cfg.HID), np.float32)
    Xp[: cfg.N] = X
    Xb = Xp.astype(BF16)
    v = Xb.reshape(cfg.NT, P, cfg.KT, cfg.KP).transpose(3, 0, 2, 1)  # [KP,NT,KT,P]
    return [np.ascontiguousarray(v[:, k * cfg.NTC:(k + 1) * cfg.NTC])
            for k in range(cfg.ncores)]


def pack_w2(cfg: Cfg2, W, a_src, a_tgt):
    WT = W.T.astype(np.float32)
    wa_s = (W.reshape(cfg.HEADS, cfg.F, cfg.HID)
            * np.asarray(a_src, np.float32).reshape(cfg.HEADS, cfg.F, 1)).sum(1)
    wa_t = (W.reshape(cfg.HEADS, cfg.F, cfg.HID)
            * np.asarray(a_tgt, np.float32).reshape(cfg.HEADS, cfg.F, 1)).sum(1)
    WA = np.concatenate([wa_s.T, wa_t.T], axis=1)
    wt = np.ascontiguousarray(
        WT.astype(BF16).reshape(cfg.KT, cfg.KP, cfg.HID).transpose(1, 0, 2))
    wa = np.ascontiguousarray(
        WA.astype(BF16).reshape(cfg.KT, cfg.KP, 2 * cfg.HEADS).transpose(1, 0, 2))
    return wt, wa


def _bcast_last(ap, n):
    import concourse.bass as bass
    lst = [list(x) for x in ap.ap] + [[0, n]]
    return bass.AP(ap.tensor, ap.offset, lst)


def _bcast_mid(ap, n):
    """Insert a 0-stride dim of size n before the last dim of an AP."""
    import concourse.bass as bass
    lst = [list(x) for x in ap.ap]
    lst = lst[:-1] + [[0, n]] + lst[-1:]
    return bass.AP(ap.tensor, ap.offset, lst)


def build_nc2(cfg: Cfg2, sched: Sched2, mode: str = "allgather",
              repeat: int = 1, cc: str = "on", skip: str = ""):
    skips = set(s for s in skip.split(",") if s)
    import concourse.bacc as bacc
    import concourse.bass as bass
    import concourse.mybir as mybir
    from concourse.tile import TileContext

    dt = mybir.dt
    H, HID, KT, KP, G = cfg.HEADS, cfg.HID, cfg.KT, cfg.KP, cfg.G

    nc = bacc.Bacc("TRN2", target_bir_lowering=False, num_devices=cfg.ncores)

    xt = nc.dram_tensor("xt", [KP, cfg.NTC, KT, P], dt.bfloat16,
                        kind="ExternalInput")
    wt = nc.dram_tensor("wt", [KP, KT, HID], dt.bfloat16, kind="ExternalInput")
    wa = nc.dram_tensor("wa", [KP, KT, 2 * H], dt.bfloat16, kind="ExternalInput")
    g1i = nc.dram_tensor("g1i", [P, sched.idxcols], dt.int16, kind="ExternalInput")
    tlocd = nc.dram_tensor("tlocd", [P, sched.TT], dt.bfloat16,
                           kind="ExternalInput")
    ohrevd = nc.dram_tensor("ohrevd", [P, sched.TT, P], dt.bfloat16,
                            kind="ExternalInput")
    iotad = nc.dram_tensor("iotad", [P, P], dt.bfloat16, kind="ExternalInput")
    avec = nc.dram_tensor("avec", [P, 1], dt.float32, kind="ExternalInput")
    out = nc.dram_tensor("out", [cfg.NW * P, HID], dt.float32,
                         kind="ExternalOutput")

    with TileContext(nc) as tc:
        with tc.tile_pool(name="const", bufs=1) as cpool, \
             tc.tile_pool(name="dram", bufs=1, space="DRAM") as dpool:
            tshard = dpool.tile([cfg.shard, cfg.row_bf], dt.bfloat16)
            if mode == "allgather4":
                tableQ = [dpool.tile([cfg.brows[q], cfg.row_bf], dt.bfloat16,
                                     addr_space="Shared", name=f"tableQ{q}")
                          for q in range(4)]
            else:
                table = dpool.tile([cfg.NPAD8, cfg.row_bf], dt.bfloat16,
                                   addr_space="Shared")
            wt_sb = cpool.tile([KP, KT, HID], dt.bfloat16)
            nc.sync.dma_start(out=wt_sb[:], in_=wt[:, :, :])
            wa_sb = cpool.tile([KP, KT, 2 * H], dt.bfloat16)
            nc.sync.dma_start(out=wa_sb[:], in_=wa[:, :, :])
            a_sb = cpool.tile([P, 1], dt.float32)
            nc.sync.dma_start(out=a_sb[:], in_=avec[:, :])
            g1i_sb = cpool.tile([P, sched.idxcols], dt.int16)
            nc.sync.dma_start(out=g1i_sb[:], in_=g1i[:, :])
            tloc_sb = cpool.tile([P, sched.TT], dt.bfloat16)
            nc.sync.dma_start(out=tloc_sb[:], in_=tlocd[:, :])
            iota_sb = cpool.tile([P, P], dt.bfloat16)
            nc.sync.dma_start(out=iota_sb[:], in_=iotad[:, :])

            # ---------------- phase 1: projection table shard ----------------
            def emit_phase1(rep):
                with tc.tile_pool(name=f"p1_{rep}", bufs=3) as xpool, \
                     tc.tile_pool(name=f"p1ps_{rep}", bufs=2, space="PSUM") as psp, \
                     tc.tile_pool(name=f"p1st_{rep}", bufs=3) as stpool:
                    if "p1" in skips:
                        stg = stpool.tile([P, 2, cfg.row_bf], dt.bfloat16,
                                          tag="stg")
                        nc.vector.memset(stg[:], 0.0)
                        nc.sync.dma_start(
                            out=tshard[0:2 * P, :].rearrange(
                                "(two p) r -> p two r", p=P),
                            in_=stg[:])
                        return
                    for j0 in range(0, cfg.NTC, 2):
                        xtile = xpool.tile([KP, 2, KT, P], dt.bfloat16, tag="x")
                        nc.scalar.dma_start(out=xtile[:],
                                            in_=xt[:, j0:j0 + 2, :, :])
                        stg = stpool.tile([P, 2, cfg.row_bf], dt.bfloat16,
                                          tag="stg")
                        stg32 = stg.bitcast(dt.float32)
                        for u in range(2):
                            ps1 = psp.tile([P, HID], dt.float32, tag="ps1")
                            ps2 = psp.tile([P, 2 * H], dt.float32, tag="ps2")
                            for ki in range(KT):
                                nc.tensor.matmul(ps1[:], xtile[:, u, ki, :],
                                                 wt_sb[:, ki, :],
                                                 start=(ki == 0),
                                                 stop=(ki == KT - 1))
                            for ki in range(KT):
                                nc.tensor.matmul(ps2[:], xtile[:, u, ki, :],
                                                 wa_sb[:, ki, :],
                                                 start=(ki == 0),
                                                 stop=(ki == KT - 1))
                            if cfg.fp8:
                                stg8 = stg.bitcast(dt.float8e4)
                                nc.scalar.copy(out=stg8[:, u, 0:HID], in_=ps1[:])
                            else:
                                nc.scalar.copy(out=stg[:, u, 0:HID], in_=ps1[:])
                            nc.vector.tensor_copy(
                                out=stg32[:, u,
                                          cfg.s_src_off:cfg.s_src_off + 2 * H],
                                in_=ps2[:])
                            nc.vector.memset(
                                stg32[:, u, cfg.s_tgt_off + H:cfg.row_f32], 0.0)
                        nc.sync.dma_start(
                            out=tshard[j0 * P:(j0 + 2) * P, :].rearrange(
                                "(two p) r -> p two r", p=P),
                            in_=stg[:])

            # ---------------- phase 1.5: resident s_tgt (hi/lo) --------------
            def emit_phase15():
                ts32 = tshard.bitcast(dt.float32)
                s_ap = ts32[:, cfg.s_tgt_off:cfg.s_tgt_off + H]
                s_ap = s_ap.rearrange("(w p) h -> p w h", p=P)
                s_all = cpool.tile([P, cfg.NW, H], dt.float32)
                nc.sync.dma_start(out=s_all[:], in_=s_ap)
                s_hilo = cpool.tile([P, cfg.NW, 2, H], dt.bfloat16)
                s_hi32 = cpool.tile([P, cfg.NW, H], dt.float32)
                nc.vector.tensor_copy(out=s_hilo[:, :, 0, :], in_=s_all[:])
                nc.vector.tensor_copy(out=s_hi32[:], in_=s_hilo[:, :, 0, :])
                nc.vector.tensor_tensor(out=s_hilo[:, :, 1, :], in0=s_all[:],
                                        in1=s_hi32[:],
                                        op=mybir.AluOpType.subtract)
                return s_hilo

            # ---------------- collective: assemble full table ----------------
            def emit_cc():
                groups = [list(range(cfg.ncores))]
                if mode == "allgather4":
                    for q in range(4):
                        if cc == "off":
                            nc.gpsimd.dma_start(out=tableQ[q][0:P, :],
                                                in_=tshard[0:P, :])
                            continue
                        lo = int(cfg.qstart[q])
                        nc.gpsimd.collective_compute(
                            "AllGather", mybir.AluOpType.bypass,
                            replica_groups=groups,
                            ins=[tshard[lo:lo + cfg.qrows[q], :].opt()],
                            outs=[tableQ[q].opt()],
                        )
                    return
                if cc == "off":
                    # timing-only ablation: satisfy write-before-read
                    nc.gpsimd.dma_start(out=table[0:P, :], in_=tshard[0:P, :])
                    return
                nc.gpsimd.collective_compute(
                    "AllGather",
                    mybir.AluOpType.bypass,
                    replica_groups=groups,
                    ins=[tshard.opt()],
                    outs=[table.opt()],
                )

            # ---------------- phase 2: window groups --------------------------
            def emit_phase2(rep, s_hilo):
                with tc.tile_pool(name=f"p2_{rep}", bufs=2) as pool, \
                     tc.tile_pool(name=f"p2ps_{rep}", bufs=2, space="PSUM") as pps:
                    for g in range(cfg.NG):
                        emit_group(pool, pps, s_hilo, g)

            def emit_group(pool, pps, s_hilo, g):
                TG = sched.TG[g]
                wsel = sched.wsel[g]
                tbase = sched.tile_base[g]
                g1t = pool.tile([P, sched.TGmax, cfg.row_bf], dt.bfloat16,
                                tag="g1t", bufs=3)
                if "gather" in skips:
                    nc.vector.memset(g1t[:, 0:1, 0:1], 0.0)
                for (b, slot_off, nslots, idxcol0) in \
                        ([] if "gather" in skips else sched.calls[g]):
                    if mode == "allgather4":
                        src_ap = tableQ[b][:, :]
                    else:
                        rows = min(cfg.NPAD8,
                                   (b + 1) * cfg.bucket) - b * cfg.bucket
                        src_ap = table[b * cfg.bucket:b * cfg.bucket + rows, :]
                    nc.gpsimd.dma_gather(
                        g1t[:, slot_off // P:(slot_off + nslots) // P, :],
                        src_ap,
                        g1i_sb[:, idxcol0:idxcol0 + nslots // 16],
                        nslots, nslots, cfg.row_bf)
                ohrev_t = pool.tile([P, sched.TGmax, P], dt.bfloat16, tag="ohrev")
                if "ohrev" in skips:
                    nc.vector.memset(ohrev_t[:, 0:1, 0:1], 0.0)
                else:
                    nc.scalar.dma_start(out=ohrev_t[:, :TG, :],
                                        in_=ohrevd[:, tbase:tbase + TG, :])
                fwd = pool.tile([P, sched.TGmax, P], dt.bfloat16, tag="fwd")
                if "fwd" in skips:
                    nc.vector.memset(fwd[:, 0:1, 0:1], 0.0)
                else:
                    nc.vector.tensor_tensor(
                        out=fwd[:, :TG, :],
                        in0=_bcast_last(tloc_sb[:, tbase:tbase + TG], P),
                        in1=_bcast_mid(iota_sb[:, :], TG),
                        op=mybir.AluOpType.is_equal)

                # s_tgt expansion into slot space (per tile); hi+lo summed in PSUM
                stgt = pps.tile([P, sched.TGmax, H], dt.float32, tag="stgt")
                if "mm" in skips:
                    nc.vector.memset(stgt[:, 0:1, 0:1], 0.0)
                for t in range(0 if "mm" in skips else TG):
                    w = G * g + wsel[t]
                    nc.tensor.matmul(stgt[:, t, :], ohrev_t[:, t, :],
                                     s_hilo[:, w, 0, :], start=True, stop=False)
                    nc.tensor.matmul(stgt[:, t, :], ohrev_t[:, t, :],
                                     s_hilo[:, w, 1, :], start=False, stop=True)

                if "vec" in skips:
                    res = pool.tile([P, G, HID], dt.float32, tag="res")
                    nc.vector.memset(res[:, 0:1, 0:1], 0.0)
                    o_ap = out[G * g * P:(G * g + G) * P, :]
                    o_ap = o_ap.rearrange("(w p) h -> p w h", p=P)
                    nc.sync.dma_start(out=o_ap, in_=res[:])
                    return
                # group-wide softmax chain
                g1t32 = g1t.bitcast(dt.float32)
                s_sum = pool.tile([P, sched.TGmax, H], dt.float32, tag="s_sum")
                s_act = pool.tile([P, sched.TGmax, H], dt.float32, tag="s_act")
                nc.vector.tensor_tensor(
                    out=s_sum[:, :TG, :], in0=stgt[:, :TG, :],
                    in1=g1t32[:, :TG, cfg.s_src_off:cfg.s_src_off + H],
                    op=mybir.AluOpType.add)
                nc.vector.scalar_tensor_tensor(
                    out=s_act[:, :TG, :], in0=s_sum[:, :TG, :], scalar=cfg.leak,
                    in1=s_sum[:, :TG, :], op0=mybir.AluOpType.mult,
                    op1=mybir.AluOpType.max)
                exp_t = pool.tile([P, sched.TGmax, H], dt.bfloat16, tag="exp_t")
                nc.scalar.activation(out=exp_t[:, :TG, :], in_=s_act[:, :TG, :],
                                     func=mybir.ActivationFunctionType.Exp)
                w_t = pool.tile([P, sched.TGmax, HID], dt.bfloat16, tag="w_t")
                if cfg.fp8:
                    projv = g1t.bitcast(dt.float8e4)[:, :TG, 0:HID]
                else:
                    projv = g1t[:, :TG, 0:HID]
                proj4 = projv.rearrange("p t (h f) -> p t h f", h=H)
                exp4 = _bcast_last(exp_t[:, :TG, :], cfg.F)
                out4 = w_t[:, :TG, :].rearrange("p t (h f) -> p t h f", h=H)
                nc.vector.tensor_tensor(out=out4, in0=proj4, in1=exp4,
                                        op=mybir.AluOpType.mult)

                # aggregation, w-major order
                agg = pps.tile([P, G, HID], dt.float32, tag="agg")
                den = pps.tile([P, G, H], dt.float32, tag="den")
                if "mm" in skips:
                    nc.vector.memset(agg[:, 0:1, 0:1], 0.0)
                    nc.vector.memset(den[:, 0:1, 0:1], 0.0)
                for wi in range(0 if "mm" in skips else G):
                    ts = [t for t in range(TG) if wsel[t] == wi]
                    for i, t in enumerate(ts):
                        st, sp = (i == 0), (i == len(ts) - 1)
                        nc.tensor.matmul(agg[:, wi, :], fwd[:, t, :],
                                         w_t[:, t, :], start=st, stop=sp)
                        nc.tensor.matmul(den[:, wi, :], fwd[:, t, :],
                                         exp_t[:, t, :], start=st, stop=sp)

                # flush both windows
                den_sb = pool.tile([P, G, H, 1], dt.float32, tag="den_sb")
                recip = pool.tile([P, G, H, 1], dt.float32, tag="recip")
                nc.vector.tensor_scalar_add(out=den_sb[:, :, :, 0], in0=den[:],
                                            scalar1=1e-16)
                nc.vector.reciprocal(out=recip[:], in_=den_sb[:])
                z = pool.tile([P, G, HID], dt.float32, tag="z")
                agg4 = agg[:].rearrange("p w (h f) -> p w h f", h=H)
                z4 = z[:].rearrange("p w (h f) -> p w h f", h=H)
                nc.vector.tensor_tensor(out=z4, in0=agg4,
                                        in1=_bcast_last(recip[:, :, :, 0], cfg.F),
                                        op=mybir.AluOpType.mult)
                res = pool.tile([P, G, HID], dt.float32, tag="res")
                nc.vector.scalar_tensor_tensor(
                    out=res[:], in0=z[:], scalar=a_sb[:, 0:1], in1=z[:],
                    op0=mybir.AluOpType.mult, op1=mybir.AluOpType.max)
                o_ap = out[G * g * P:(G * g + G) * P, :]
                o_ap = o_ap.rearrange("(w p) h -> p w h", p=P)
                nc.sync.dma_start(out=o_ap, in_=res[:])

            for rep in range(repeat):
                emit_phase1(rep)
                s_hilo = emit_phase15()
                if rep == 0:
                    emit_cc()
                emit_phase2(rep, s_hilo)
                if rep < repeat - 1:
                    tc.strict_bb_all_engine_barrier()

    nc.compile()
    return nc


def prepare2(cfg: Cfg2, inputs, mode: str = "allgather"):
    X = np.asarray(inputs["in_nodes_features"], np.float32)
    ei = np.asarray(inputs["edge_index"], np.int64)
    W = np.asarray(inputs["W"], np.float32)
    b_lin = np.asarray(inputs["b_lin"], np.float32)
    a_src = np.asarray(inputs["a_src"], np.float32)
    a_tgt = np.asarray(inputs["a_tgt"], np.float32)
    bias = np.asarray(inputs["bias"], np.float32)
    prelu_a = float(np.asarray(inputs["prelu_a"], np.float32))

    assert np.all(b_lin == 0) and np.all(bias == 0), "nonzero bias unsupported"
    assert 0.0 <= prelu_a <= 1.0, "prelu_a outside [0,1] unsupported"

    src, trg = ei[0], ei[1]
    core_of = trg // cfg.shard
    win_of = (trg % cfg.shard) // P
    if mode == "allgather4":
        sk = src // cfg.shard
        sj = src % cfg.shard
        qs = np.asarray(cfg.qstart, np.int64)
        qr = np.asarray(cfg.qrows, np.int64)
        buck_of = np.searchsorted(qs, sj, side="right") - 1
        eidx = sk * qr[buck_of] + (sj - qs[buck_of])
    else:
        buck_of = src // cfg.bucket
        eidx = src - buck_of * cfg.bucket
    counts = np.zeros((cfg.ncores, cfg.NW, cfg.NB), np.int64)
    for k in range(cfg.ncores):
        m = core_of == k
        counts[k] = np.bincount(
            win_of[m] * cfg.NB + buck_of[m],
            minlength=cfg.NW * cfg.NB).reshape(cfg.NW, cfg.NB)
    sched = Sched2(cfg, counts)

    xts = pack_xt2(cfg, X)
    wtp, wap = pack_w2(cfg, W, a_src, a_tgt)
    av = np.full((P, 1), prelu_a, np.float32)
    iota = np.broadcast_to(np.arange(P, dtype=np.float32), (P, P)).astype(BF16)
    iota = np.ascontiguousarray(iota)

    in_maps = []
    for k in range(cfg.ncores):
        g1i_k, tloc_k, ohrev_k = prep_core2(cfg, sched, eidx, buck_of, trg, k)
        in_maps.append({
            "xt": xts[k], "wt": wtp, "wa": wap, "g1i": g1i_k,
            "tlocd": tloc_k, "ohrevd": ohrev_k, "iotad": iota, "avec": av,
        })

    def assemble(core_outs):
        full = np.concatenate(
            [np.asarray(o["out"], np.float32) for o in core_outs], axis=0)
        return full[: cfg.N]

    return sched, in_maps, assemble


_BUILT2 = {}


MODE = "allgather4"


def kernel(**inputs):
    from concourse.bass_utils import run_bass_kernel_spmd

    cfg = Cfg2()
    sched, in_maps, assemble = prepare2(cfg, inputs, mode=MODE)
    key = (tuple(sched.TG), sched.idxcols)
    if key not in _BUILT2:
        _BUILT2[key] = build_nc2(cfg, sched, mode=MODE)
    nc = _BUILT2[key]
    res = run_bass_kernel_spmd(nc, in_maps, core_ids=list(range(cfg.ncores)))
    return assemble(res.results)


# --- back-compat aliases so the existing test.py harness keeps working ---
Cfg = Cfg2


def prepare(cfg, inputs):
    return prepare2(cfg, inputs, mode=MODE)


def build_nc(cfg, sched, phases="full", skip="", repeat=1):
    return build_nc2(cfg, sched, mode=MODE, repeat=repeat, skip=skip)
